# revision 4
# baseline (speedup 1.0000x reference)
"""GNN mean-aggregation + 2-layer MLP on 8 Trainium2 NeuronCores, v2.

Reference computation:
    rows = [i;j], cols = [j;i]                      (symmetrized COO)
    agg[n]  = mean over entries (n, c) of conical[c]   (deg clamped to 1)
    out     = relu([radial | agg] @ W1 + b1) @ W2 + b2

Design (v2, pad-free gather + PE segmented reduce):
  The gather of conical[cols] dominates. dma_gather (InstDMAGatherAnt)
  costs ~2.0-2.6ns per descriptor regardless of payload size (32B..256B)
  -- the 4 SWDGE rings drain at ~7.8ns/descriptor each, in parallel -- so
  the only lever is descriptor COUNT. v1 padded each node's neighbor list
  per (group, class) to a shared width -> 1.39x descriptors. v2 is
  pad-free: entries are sorted by (group, class, local-node) and chunked
  into 128s with no per-node alignment (last-chunk-per-(group,class)
  padding ~3% + cross-core SPMD max-padding ~3% only).

  The per-node reduction that padding used to provide moves to the
  Tensor engine: for each 128-entry chunk, a [128e x 128n] 0/1 segment
  matrix (host-built, streamed bf16 from HBM at ~136MB/core on the
  scalar HWDGE queue, ~0.4ms fully hidden under the gather) is the
  matmul stationary; the gathered [128e x 16f] bf16 values are the
  moving operand; PSUM accumulates agg sums per 128-node tile. Chunks
  may straddle tiles: one matmul per overlapped tile (foreign entries
  have zero rows). All matmuls of one PSUM region are issued
  contiguously -- interleaving accumulation groups corrupts results on
  HW even though element-wise has_written suggests it should work.

  conical is gathered in bf16 (table [12502, 128] bf16: stripe s packs
  nodes 8s..8s+7; class m = col%8 selects byte offset m*32 at 256B
  stride; idx = col//8 fits int16). rel err ~3e-4 vs the 2e-2 gate.

  Pipeline: 9 super-blocks (3 MLP groups each; last=1) double-buffered;
  per (sb, class) the gather is split into TWO instructions so the PE
  can start a super-block's matmuls after only half its gathers landed
  (the tile framework tracks slice-level ranges). Segment slabs are
  fetched per half-group (2 tiles) with bufs=3 prefetch. Measured
  ~1.25ms vs 1.73-1.83ms for v1; gather engine busy ~860us (=
  424k descriptors at the ring-drain floor), remainder is pipeline
  head/tail and residual inter-super-block bubbles.

  Nodes are deg-snake assigned to cores (balances entries/core to
  ~0.01%). MLP identical to v1: PE transpose to feature-major,
  stationary weights, ones-row bias trick, node-major store, host
  inverse-permutes.
"""

import numpy as np

N_CORES = 8
P = 128
FH = 16
F = 32
HID = 128
CLS = 8
GT = 4  # tiles per MLP group
SBG = 5  # groups per gather super-block
MAX_W = 124  # max 128-idx chunks per gather instruction


# ---------------------------------------------------------------- host prep


def _host_prep(x, edge_index):
    N = x.shape[0]
    NPC = (N + N_CORES - 1) // N_CORES  # 12500
    NT = (NPC + P - 1) // P  # 98 tiles/core
    NL = NT * P  # 12544 local slots
    NG = (NT + GT - 1) // GT  # 25 groups
    TROWS = (N + CLS - 1) // CLS + 2  # 12502 (2 zero stripes)
    PAD_STRIPE = TROWS - 2

    i = edge_index[0].astype(np.int64)
    j = edge_index[1].astype(np.int64)
    rows = np.concatenate([i, j])
    cols = np.concatenate([j, i])
    deg = np.bincount(rows, minlength=N)

    # deg-snake core assignment
    order_by_deg = np.argsort(-deg, kind="stable")
    snake = np.tile(
        np.concatenate([np.arange(N_CORES), np.arange(N_CORES)[::-1]]),
        N // (2 * N_CORES) + 1,
    )[:N]
    core_of = np.empty(N, np.int8)
    lslot = np.empty(N, np.int64)
    core_of[order_by_deg] = snake
    nodemap = np.full((N_CORES, NL), -1, np.int64)  # local slot -> global node
    for c in range(N_CORES):
        nodes_c = order_by_deg[snake == c]
        lslot[nodes_c] = np.arange(len(nodes_c))
        nodemap[c, : len(nodes_c)] = nodes_c

    # per (core, group, class) entry counts -> shared chunk structure
    ln_all = lslot[rows]
    cls_all = cols % CLS
    grp_all = ln_all // (GT * P)
    c_all = core_of[rows].astype(np.int64)
    key = (c_all * NG + grp_all) * CLS + cls_all
    cnt = np.bincount(key, minlength=N_CORES * NG * CLS).reshape(N_CORES, NG, CLS)
    CH = np.maximum((cnt + P - 1) // P, 1).max(axis=0)  # [NG, CLS] shared chunks

    # shared stream layout: for sb, for m, for g in sb: CH[g,m] chunks
    _szs = [3, 3, 3, 3, 3, 3, 3, 3, 1]
    sbs = []
    _b = 0
    for _sz in _szs:
        sbs.append(list(range(_b, min(_b + _sz, NG))))
        _b += _sz
    assert _b == NG
    chunk_of = {}  # (g, m) -> global chunk start
    gcol = 0
    sb_meta = []  # per sb: (chunk_start, per-class list of (col0, nch))
    for sb in sbs:
        cstart = gcol
        percls = []
        for m in range(CLS):
            col0 = gcol
            for g in sb:
                chunk_of[(g, m)] = gcol
                gcol += int(CH[g, m])
            percls.append((col0, gcol - col0))
        sb_meta.append((cstart, percls))
    NCH = gcol

    # per-core streams
    eorder_key = (c_all * NG + grp_all) * (CLS * NL) + cls_all * NL + ln_all
    eo = np.argsort(eorder_key, kind="stable")
    ln_s = ln_all[eo]
    stripe_s = (cols[eo] // CLS).astype(np.int32)
    c_s = c_all[eo]
    grp_s = grp_all[eo]
    cls_s = cls_all[eo]

    idx_full = np.full((N_CORES, NCH * P), PAD_STRIPE, np.int32)
    nid_full = np.full((N_CORES, NCH, P), -1.0, np.float32)
    # boundaries per (c, g, m)
    bkey = (c_all * NG + grp_all) * CLS + cls_all
    border = np.zeros(N_CORES * NG * CLS + 1, np.int64)
    border[1:] = np.cumsum(np.bincount(bkey, minlength=N_CORES * NG * CLS))
    t0a = np.full((N_CORES, NCH), 10**9, np.int64)
    t1a = np.full((N_CORES, NCH), -1, np.int64)
    for c in range(N_CORES):
        for g in range(NG):
            for m in range(CLS):
                b = (c * NG + g) * CLS + m
                lo, hi = border[b], border[b + 1]
                nch = int(CH[g, m])
                base = chunk_of[(g, m)] * P
                idx_full[c, base : base + (hi - lo)] = stripe_s[lo:hi]
                nid = nid_full[c, chunk_of[(g, m)] : chunk_of[(g, m)] + nch]
                nid.reshape(-1)[: hi - lo] = ln_s[lo:hi].astype(np.float32)
                # per-chunk tile span
                tl = ln_s[lo:hi] // P
                for ch in range((hi - lo + P - 1) // P):
                    gc = chunk_of[(g, m)] + ch
                    seg = tl[ch * P : (ch + 1) * P]
                    if len(seg):
                        t0a[c, gc] = min(t0a[c, gc], int(seg.min()))
                        t1a[c, gc] = max(t1a[c, gc], int(seg.max()))
    t0u = np.where(t0a.min(axis=0) == 10**9, -1, t0a.min(axis=0))
    t1u = t1a.max(axis=0)

    # matmul emission order (shared): per (g, t): all chunk-matmuls contiguous
    # so each PSUM region's accumulation group is uninterrupted
    mm = []  # (g, gchunk, tile, first, last) in issue order
    NT_total = NT
    for g in range(NG):
        gs = min(GT, NT_total - g * GT)
        for ti in range(gs):
            t = g * GT + ti
            ks = []
            for m in range(CLS):
                for ch in range(int(CH[g, m])):
                    gc = chunk_of[(g, m)] + ch
                    if t1u[gc] >= 0 and t0u[gc] <= t <= t1u[gc]:
                        ks.append(gc)
            if not ks:  # ensure region is initialized
                ks.append(chunk_of[(g, 0)])
            for kidx, gc in enumerate(ks):
                mm.append((g, gc, t, kidx == 0, kidx == len(ks) - 1))
    NMM = len(mm)

    # host-built segment matrices, bf16 [P e, NMM*P n]
    import ml_dtypes

    gcs = np.array([r[1] for r in mm], np.int64)
    ts = np.array([r[2] for r in mm], np.int64)
    segs = np.empty((N_CORES, P, NMM * P), np.dtype(ml_dtypes.bfloat16))
    for c in range(N_CORES):
        vals = nid_full[c, gcs, :] - (ts[:, None] * P)  # [NMM, P(e)]
        kk, ee = np.nonzero((vals >= 0) & (vals < P))
        sc = np.zeros((NMM, P, P), np.float32)  # [k, e, n]
        sc[kk, ee, vals[kk, ee].astype(np.int64)] = 1.0
        segs[c] = sc.transpose(1, 0, 2).reshape(P, NMM * P)

    # wrapped idx slabs (per sb, per instruction): [16, n/16] replicated x8
    idxw = np.empty((N_CORES, P, NCH * 8), np.int16)
    for c in range(N_CORES):
        w = idx_full[c].reshape(-1, 16).T.astype(np.int16)  # [16, NCH*8]
        idxw[c] = np.tile(w, (8, 1))

    # gather instructions (shared): per sb, per m: split (col0, nch) by MAX_W
    ginstr = []  # (sbi, m, chunk0, nch, queue)
    for sbi, (cstart, percls) in enumerate(sb_meta):
        for half in range(2):
            for m in range(CLS):
                col0, nch = percls[m]
                h0 = (nch + 1) // 2 if half else 0
                h1 = nch if half else (nch + 1) // 2
                off = h0
                while off < h1:
                    take = min(MAX_W, h1 - off)
                    ginstr.append((sbi, m, col0 + off, take, m % 4))
                    off += take

    # invdeg expanded [P, NG*GT*FH]
    invdeg_exp = np.ones((N_CORES, P, NT * FH), np.float32)
    radial = np.zeros((N_CORES, FH, NL), np.float32)
    for c in range(N_CORES):
        nm = nodemap[c]
        v = nm >= 0
        iv = np.ones(NL, np.float32)
        iv[v] = (1.0 / np.maximum(deg[nm[v]], 1)).astype(np.float32)
        # [p, g*64 + ti*16 + f] = iv[(g*4+ti)*128 + p]
        invdeg_exp[c] = np.repeat(iv.reshape(NT, P).T, FH, axis=1).reshape(
            P, NT * FH
        )
        r = np.zeros((NL, FH), np.float32)
        r[v] = x[nm[v], :FH]
        radial[c] = r.T

    # bf16 conical table [TROWS, 128]
    import ml_dtypes

    tbl = np.zeros((TROWS * CLS, FH), np.float32)
    tbl[:N] = x[:, FH:F]
    table = tbl.reshape(TROWS, CLS * FH).astype(ml_dtypes.bfloat16)

    return dict(
        NL=NL, NT=NT, NG=NG, NCH=NCH, NMM=NMM, TROWS=TROWS,
        CH=CH, sb_meta=sb_meta, sbs=sbs, chunk_of=chunk_of, mm=mm,
        ginstr=ginstr, idxw=idxw, segs=segs, invdeg_exp=invdeg_exp,
        radial=radial, table=table, nodemap=nodemap,
    )


# ------------------------------------------------------------- bass program


def _dma_gather_raw(nc, out_ap, in_ap, idxs_ap, num_idxs, queue_num):
    """InstDMAGatherAnt non-transpose, elem 16 bf16 (32B) at 256B stride."""
    from concourse import mybir

    eng = nc.gpsimd
    elem_step = in_ap.ap[0][0]
    stride_bytes = elem_step * mybir.dt.size(in_ap.dtype)
    assert stride_bytes == 256
    return eng.add_instruction(
        mybir.InstDMAGatherAnt(
            name=eng.bass.get_next_instruction_name(),
            ins=[
                *eng.lower_ap_dma(in_ap, for_custom_bir_dma=True),
                eng.lower_ap(idxs_ap),
                eng.lower_val_access(eng.to_reg(num_idxs)),
            ],
            outs=[eng.lower_ap(out_ap)],
            transpose=False,
            num_idxs=num_idxs,
            elem_size=FH,
            stride_bytes_256=1,
            gen_mode=0,
            single_packet=False,
            queue_num=queue_num,
            sbuf_tokens_per_rank=0,
            sbuf_free_dim_per_rank=0,
            sbuf_free_dim_pad_per_rank=0,
            sbuf_byte_offset=0,
        )
    )


def build_program(prep):
    import concourse.tile as tile
    from concourse import bacc, mybir

    f32 = mybir.dt.float32
    bf16 = mybir.dt.bfloat16
    i16 = mybir.dt.int16
    AF = mybir.ActivationFunctionType
    ALU = mybir.AluOpType

    NL, NT, NG, NCH, NMM, TROWS = (
        prep["NL"], prep["NT"], prep["NG"], prep["NCH"], prep["NMM"],
        prep["TROWS"],
    )
    CH, sb_meta, sbs, chunk_of, mm, ginstr = (
        prep["CH"], prep["sb_meta"], prep["sbs"], prep["chunk_of"],
        prep["mm"], prep["ginstr"],
    )

    nc = bacc.Bacc(None, num_swdge_queues=4)
    table = nc.dram_tensor("table", [TROWS, CLS * FH], bf16, kind="ExternalInput")
    idxw = nc.dram_tensor("idxw", [P, NCH * 8], i16, kind="ExternalInput")
    segs = nc.dram_tensor("segs", [P, NMM * P], bf16, kind="ExternalInput")
    invdeg = nc.dram_tensor("invdeg", [P, NT * FH], f32, kind="ExternalInput")
    radial = nc.dram_tensor("radial", [FH, NL], f32, kind="ExternalInput")
    ident = nc.dram_tensor("ident", [P, P], f32, kind="ExternalInput")
    w1a = nc.dram_tensor("w1a", [FH, HID], f32, kind="ExternalInput")
    w1b = nc.dram_tensor("w1b", [FH, HID], f32, kind="ExternalInput")
    w2 = nc.dram_tensor("w2", [HID, F], f32, kind="ExternalInput")
    b1 = nc.dram_tensor("b1", [HID, 1], f32, kind="ExternalInput")
    b2 = nc.dram_tensor("b2", [1, F], f32, kind="ExternalInput")
    out = nc.dram_tensor("out", [NL, F], f32, kind="ExternalOutput")

    # per-sb chunk extents for G slabs
    sb_ch0 = [cstart for (cstart, _) in sb_meta]
    sb_nch = []
    for sbi, (cstart, percls) in enumerate(sb_meta):
        end = percls[-1][0] + percls[-1][1]
        sb_nch.append(end - cstart)
    max_sb_nch = max(sb_nch)

    with tile.TileContext(nc) as tc:
        with (
            tc.tile_pool(name="res", bufs=1) as res,
            tc.tile_pool(name="gslab", bufs=2) as gslab,
            tc.tile_pool(name="islab", bufs=2) as islab,
            tc.tile_pool(name="segsl", bufs=3) as segsl,
            tc.tile_pool(name="mlp", bufs=3) as mlp,
            tc.tile_pool(name="psA", bufs=2, space="PSUM") as psA,
            tc.tile_pool(name="psB", bufs=2, space="PSUM") as psB,
        ):
            ident_sb = res.tile([P, P], f32)
            nc.sync.dma_start(out=ident_sb[:], in_=ident[:])
            invdeg_sb = res.tile([P, NT * FH], f32)
            nc.sync.dma_start(out=invdeg_sb[:], in_=invdeg[:])
            w1a_sb = res.tile([FH, HID], f32)
            nc.sync.dma_start(out=w1a_sb[:], in_=w1a[:])
            w1b_sb = res.tile([FH, HID], f32)
            nc.sync.dma_start(out=w1b_sb[:], in_=w1b[:])
            w2_sb = res.tile([HID, F], f32)
            nc.sync.dma_start(out=w2_sb[:], in_=w2[:])
            b1_sb = res.tile([HID, 1], f32)
            nc.sync.dma_start(out=b1_sb[:], in_=b1[:])
            b2_sb = res.tile([1, F], f32)
            nc.sync.dma_start(out=b2_sb[:], in_=b2[:])
            ones_sb = res.tile([1, P], f32)
            nc.vector.memset(ones_sb[:], 1.0)

            # mm issue-order index per (group, tile)
            mm_by_gt = {}
            for k, rec in enumerate(mm):
                mm_by_gt.setdefault((rec[0], rec[2]), []).append((k,) + rec[1:])
            gt_k0 = {gt: min(k for k, *_ in v) for gt, v in mm_by_gt.items()}
            gt_nmm = {gt: len(v) for gt, v in mm_by_gt.items()}
            max_gtnmm = max(gt_nmm.values())
            hg_sizes = []
            for g in range(NG):
                gs_ = min(GT, NT - g * GT)
                for h0 in range(0, gs_, 2):
                    hts = [g * GT + ti for ti in range(h0, min(h0 + 2, gs_))]
                    hg_sizes.append(
                        gt_k0[(g, hts[-1])] + gt_nmm[(g, hts[-1])] - gt_k0[(g, hts[0])]
                    )
            max_hg = max(hg_sizes)

            for sbi, sb in enumerate(sbs):
                ch0 = sb_ch0[sbi]
                nch = sb_nch[sbi]
                G = gslab.tile([P, max_sb_nch * FH], bf16, tag="G")
                ID = islab.tile([P, max_sb_nch * 8], i16, tag="ID")
                nc.sync.dma_start(
                    out=ID[:, : nch * 8], in_=idxw[:, ch0 * 8 : (ch0 + nch) * 8]
                )
                for sbj, m, c0, w, q in ginstr:
                    if sbj != sbi:
                        continue
                    lc = c0 - ch0
                    _dma_gather_raw(
                        nc,
                        G[:, lc * FH : (lc + w) * FH].rearrange(
                            "p (c f) -> p c f", f=FH
                        ),
                        table[:, m * FH : (m + 1) * FH],
                        ID[:, lc * 8 : (lc + w) * 8],
                        P * w,
                        queue_num=q,
                    )

                for g in sb:
                    gs = min(GT, NT - g * GT)
                    nb = gs * P
                    agg_ps = psA.tile([P, GT * FH], f32, tag="agg")
                    for h0 in range(0, gs, 2):
                        hts = [g * GT + ti for ti in range(h0, min(h0 + 2, gs))]
                        k0 = gt_k0[(g, hts[0])]
                        kend = gt_k0[(g, hts[-1])] + gt_nmm[(g, hts[-1])]
                        nk = kend - k0
                        SG = segsl.tile([P, max_hg * P], bf16, tag="SG")
                        nc.scalar.dma_start(
                            out=SG[:, : nk * P],
                            in_=segs[:, k0 * P : (k0 + nk) * P],
                        )
                        for t in hts:
                            ti = t - g * GT
                            for k, gc, tt, fi, la in mm_by_gt[(g, t)]:
                                lc = gc - ch0
                                nc.tensor.matmul(
                                    agg_ps[:, ti * FH : (ti + 1) * FH],
                                    SG[:, (k - k0) * P : (k - k0 + 1) * P],
                                    G[:, lc * FH : (lc + 1) * FH],
                                    start=fi,
                                    stop=la,
                                )

                    agg_sb = mlp.tile([P, GT * FH], f32, tag="agg_sb")
                    nc.scalar.activation(
                        agg_sb[:, : gs * FH], agg_ps[:, : gs * FH], AF.Copy
                    )
                    nc.vector.tensor_tensor(
                        out=agg_sb[:, : gs * FH],
                        in0=agg_sb[:, : gs * FH],
                        in1=invdeg_sb[
                            :, g * GT * FH : (g * GT + gs) * FH
                        ],
                        op=ALU.mult,
                    )
                    tr_ps = psB.tile([FH, GT * P], f32, tag="tr")
                    for ti in range(gs):
                        nc.tensor.transpose(
                            out=tr_ps[:, ti * P : (ti + 1) * P],
                            in_=agg_sb[:, ti * FH : (ti + 1) * FH],
                            identity=ident_sb[:],
                        )
                    agg_f = mlp.tile([FH, GT * P], f32, tag="agg_f")
                    nc.scalar.activation(agg_f[:, :nb], tr_ps[:, :nb], AF.Copy)
                    rad_sb = mlp.tile([FH, GT * P], f32, tag="rad")
                    nc.sync.dma_start(
                        out=rad_sb[:, :nb],
                        in_=radial[:, g * GT * P : g * GT * P + nb],
                    )
                    h_ps = psA.tile([HID, GT * P], f32, tag="h")
                    nc.tensor.matmul(
                        h_ps[:, :nb], w1a_sb[:], rad_sb[:, :nb],
                        start=True, stop=False,
                    )
                    nc.tensor.matmul(
                        h_ps[:, :nb], w1b_sb[:], agg_f[:, :nb],
                        start=False, stop=True,
                    )
                    h_sb = mlp.tile([HID, GT * P], f32, tag="h_sb")
                    nc.scalar.activation(
                        h_sb[:, :nb], h_ps[:, :nb], AF.Relu, bias=b1_sb[:, :1]
                    )
                    o_ps = psB.tile([P, GT * F], f32, tag="o")
                    for ti in range(gs):
                        nc.tensor.matmul(
                            o_ps[:, ti * F : (ti + 1) * F],
                            ones_sb[:],
                            b2_sb[:],
                            start=True,
                            stop=False,
                        )
                        nc.tensor.matmul(
                            o_ps[:, ti * F : (ti + 1) * F],
                            h_sb[:, ti * P : (ti + 1) * P],
                            w2_sb[:],
                            start=False,
                            stop=True,
                        )
                    o_sb = mlp.tile([P, GT * F], f32, tag="o_sb")
                    nc.scalar.activation(
                        o_sb[:, : gs * F], o_ps[:, : gs * F], AF.Copy
                    )
                    for ti in range(gs):
                        r0 = (g * GT + ti) * P
                        nc.scalar.dma_start(
                            out=out[r0 : r0 + P, :],
                            in_=o_sb[:, ti * F : (ti + 1) * F],
                        )
    return nc


# ------------------------------------------------------------------ driver


def _run(x, edge_index, W1, b1, W2, b2, trace=False):
    from concourse.bass_utils import run_bass_kernel_spmd

    x = np.ascontiguousarray(np.asarray(x), np.float32)
    edge_index = np.ascontiguousarray(np.asarray(edge_index), np.int32)
    W1 = np.ascontiguousarray(np.asarray(W1), np.float32)
    W2 = np.ascontiguousarray(np.asarray(W2), np.float32)
    b1v = np.ascontiguousarray(np.asarray(b1), np.float32).reshape(HID, 1)
    b2v = np.ascontiguousarray(np.asarray(b2), np.float32).reshape(1, F)

    prep = _host_prep(x, edge_index)
    nc = build_program(prep)
    if not nc.is_finalized():
        nc.finalize()

    in_maps = []
    for c in range(N_CORES):
        in_maps.append(
            {
                "table": prep["table"],
                "idxw": np.ascontiguousarray(prep["idxw"][c]),
                "segs": np.ascontiguousarray(prep["segs"][c]),
                "invdeg": np.ascontiguousarray(prep["invdeg_exp"][c]),
                "radial": np.ascontiguousarray(prep["radial"][c]),
                "ident": np.eye(P, dtype=np.float32),
                "w1a": np.ascontiguousarray(W1[:FH]),
                "w1b": np.ascontiguousarray(W1[FH:]),
                "w2": W2,
                "b1": b1v,
                "b2": b2v,
            }
        )
    br = run_bass_kernel_spmd(nc, in_maps, list(range(N_CORES)), trace=trace)

    N = x.shape[0]
    result = np.empty((N, F), np.float32)
    for c in range(N_CORES):
        shard = np.asarray(br.results[c]["out"])
        nm = prep["nodemap"][c]
        v = nm >= 0
        result[nm[v]] = shard[v]
    return result, br


def kernel(x, edge_index, W1, b1, W2, b2):
    result, _ = _run(x, edge_index, W1, b1, W2, b2)
    return result
